# revision 21
# baseline (speedup 1.0000x reference)
"""Trainium2 Bass kernel for the NeRF renderer + distortion loss.

Layout: pure data-parallel over rays. 1024 rays / 8 cores = 128 rays per core
= exactly the 128 SBUF partitions; the N=576 samples live on the free dim.

Per-core pipeline (DVE = vector engine, ACT = scalar engine, Pool = gpsimd):

  constants (deltas / 1/z / z_lin) are broadcast-DMA'd to [128,576]
  (partition-stride-0 source APs), ordered by when the pipeline needs them

  x     = sigma * delta            DVE
  t     = exp(-x)                  ACT  (HW exp underflows cleanly to 0
                                   for x=-1e10, verified, so no clamp)
  trans = scan: s = t*s + eps      DVE tensor_tensor_scan (exclusive via
                                   writing out to cols 1..N of an N+1 tile;
                                   differs from cumprod(t+eps) by <1e-9)
  wneg  = (t-1)*trans = -w         DVE stt in two halves interleaved with
                                   the scan halves (fills DVE idle gaps),
                                   accums -> -W in two host-summed parts
  wtn   = wneg*z_lin  = -w*t       DVE stt, accum -> -T
  Pn    = scan: s = (wneg+s)+eps   DVE (= -P inclusive cumsum, eps leak <6e-8)
  r1    = sum(wtn*Pn)              DVE stt accum
  -img_r= sum(wneg*rgb_r)          DVE stt accum
  r3    = sum(wneg*wtn)            DVE stt accum
  -invd = sum(wneg*invz)           DVE stt accum
  -img_g, -img_b                   product on Pool (tensor_mul), summed on
                                   ACT (activation Copy + accum_out) — the
                                   two idle engines absorb two reductions

Distortion loss per ray (z_lin sorted ascending):
  sum_ij w_i w_j |t_i - t_j| = 2*(2*sum(w*t*P) - W*T - sum(w*w*t))
combined on host from the per-ray scalars; final mean over rays on host
(the "all-reduce" of the sharding hint happens in the gather).
"""

import sys

if "/opt/trn_rl_repo" not in sys.path:
    sys.path.insert(0, "/opt/trn_rl_repo")

import numpy as np

N_INNER, N_OUTER = 384, 192
N = N_INNER + N_OUTER          # 576 samples per ray
R = 1024                       # rays
NCORES = 8
RS = R // NCORES               # 128 rays per core == SBUF partitions
EPS = 1e-10

_BUILT = None                  # cached compiled Bass module


def _constants():
    """[3, 576] f32: rows = (deltas, 1/z, z_lin); matches reference's
    jnp.logspace/linspace to <=1 ulp."""
    zi = np.power(10.0, np.linspace(-1.2, 0.0, N_INNER)).astype(np.float32)
    zo = np.power(10.0, np.linspace(0.0, 2.0, N_OUTER)).astype(np.float32)
    z = np.concatenate([zi, zo]).astype(np.float32)
    zl = np.concatenate(
        [np.linspace(-1.2, 0.0, N_INNER), np.linspace(0.0, 2.0, N_OUTER)]
    ).astype(np.float32)
    zlin = ((zl + np.float32(1.2)) / np.float32(3.2)).astype(np.float32)
    deltas = np.concatenate([np.diff(z), np.array([1e10], np.float32)]).astype(
        np.float32
    )
    invz = (np.float32(1.0) / z).astype(np.float32)
    return np.stack([deltas, invz, zlin]).astype(np.float32)


def _build():
    import concourse.bass as bass
    import concourse.bacc as bacc
    import concourse.tile as tile
    from concourse import mybir

    Alu = mybir.AluOpType
    AF = mybir.ActivationFunctionType
    F32 = mybir.dt.float32

    nc = bacc.Bacc("TRN2", target_bir_lowering=False, debug=False)

    d_sig = nc.dram_tensor("sig", [RS, N], F32, kind="ExternalInput")
    d_rgb = nc.dram_tensor("rgb", [RS, 3 * N], F32, kind="ExternalInput")
    d_cst = nc.dram_tensor("cst", [3, N], F32, kind="ExternalInput")
    d_out = nc.dram_tensor("out", [RS, 9], F32, kind="ExternalOutput")

    with tile.TileContext(nc) as tc:
        with (
            tc.tile_pool(name="p", bufs=1) as p,
            tc.tile_pool(name="scr", bufs=2) as scrp,
        ):
            s_sig = p.tile([RS, N], F32)
            s_rgb = p.tile([RS, 3 * N], F32)
            s_delta = p.tile([RS, N], F32)
            s_invz = p.tile([RS, N], F32)
            s_zlin = p.tile([RS, N], F32)
            s_eps = p.tile([RS, N], F32)
            s_x = p.tile([RS, N], F32)
            s_t = p.tile([RS, N], F32)
            s_transw = p.tile([RS, N + 1], F32)
            s_wneg = p.tile([RS, N], F32)
            s_wtn = p.tile([RS, N], F32)
            s_pn = p.tile([RS, N], F32)
            s_pack = p.tile([RS, 9], F32)

            # ---- loads: one serial DMA track; order = criticality.
            # delta+sigma gate the DVE chain; zlin is needed mid-chain;
            # rgb feeds the Pool image products; invz feeds the last accum ----
            H = N // 2

            def bcast_cols(tl, row, lo, hi):
                r = d_cst[row : row + 1, lo:hi]
                nc.sync.dma_start(
                    out=tl[:, lo:hi],
                    in_=bass.AP(
                        tensor=r.tensor, offset=r.offset, ap=[[0, RS], [1, hi - lo]]
                    ),
                )

            # One serial HWDGE dispatch device (~630ns/DMA), one shared
            # transfer track — so DMA count and order are what matter.
            # delta full first (overlaps the next dispatches), sigma in two
            # halves so the first mult starts earlier, z_lin mid-chain, rgb
            # planes for the image products, invz for the last accum.
            bcast_cols(s_delta, 0, 0, N)
            nc.sync.dma_start(out=s_sig[:, 0:H], in_=d_sig[:, 0:H])
            nc.sync.dma_start(out=s_sig[:, H:N], in_=d_sig[:, H:N])
            bcast_cols(s_zlin, 2, 0, N)
            nc.sync.dma_start(out=s_rgb[:, 0 : 2 * N], in_=d_rgb[:, 0 : 2 * N])
            nc.sync.dma_start(out=s_rgb[:, 2 * N : 3 * N], in_=d_rgb[:, 2 * N : 3 * N])
            bcast_cols(s_invz, 1, 0, N)
            nc.gpsimd.memset(s_eps[:], EPS)
            nc.gpsimd.memset(s_transw[:, 0:1], 1.0)

            # ---- compute ----
            # head in two halves: mult/exp/scan pipeline across DVE+ACT,
            # scan chained via initial=prev last column
            nc.vector.tensor_mul(s_x[:, 0:H], s_sig[:, 0:H], s_delta[:, 0:H])
            nc.vector.tensor_mul(s_x[:, H:N], s_sig[:, H:N], s_delta[:, H:N])
            nc.scalar.activation(s_t[:, 0:H], s_x[:, 0:H], AF.Exp, bias=0.0, scale=-1.0)
            nc.scalar.activation(s_t[:, H:N], s_x[:, H:N], AF.Exp, bias=0.0, scale=-1.0)
            nc.vector.tensor_tensor_scan(
                out=s_transw[:, 1 : H + 1],
                data0=s_t[:, 0:H],
                data1=s_eps[:, 0:H],
                initial=1.0,
                op0=Alu.mult,
                op1=Alu.add,
            )
            nc.vector.scalar_tensor_tensor(
                out=s_wneg[:, 0:H],
                in0=s_t[:, 0:H],
                scalar=1.0,
                in1=s_transw[:, 0:H],
                op0=Alu.subtract,
                op1=Alu.mult,
                accum_out=s_pack[:, 6:7],  # -W (first half)
            )
            nc.vector.tensor_tensor_scan(
                out=s_transw[:, H + 1 : N + 1],
                data0=s_t[:, H:N],
                data1=s_eps[:, H:N],
                initial=s_transw[:, H : H + 1],
                op0=Alu.mult,
                op1=Alu.add,
            )
            nc.vector.scalar_tensor_tensor(
                out=s_wneg[:, H:N],
                in0=s_t[:, H:N],
                scalar=1.0,
                in1=s_transw[:, H:N],
                op0=Alu.subtract,
                op1=Alu.mult,
                accum_out=s_pack[:, 7:8],  # -W (second half)
            )
            nc.vector.scalar_tensor_tensor(
                out=s_wtn[:],
                in0=s_wneg[:],
                scalar=1.0,
                in1=s_zlin[:],
                op0=Alu.mult,
                op1=Alu.mult,
                accum_out=s_pack[:, 8:9],  # -T
            )
            nc.vector.tensor_tensor_scan(
                out=s_pn[:],
                data0=s_wneg[:],
                data1=s_eps[:],
                initial=0.0,
                op0=Alu.add,
                op1=Alu.add,
            )

            def accum(in0, in1, col):
                # fused product+sum on DVE: out = in0*in1, accum_out = sum
                s_scr = scrp.tile([RS, N], F32, tag="scr", name=f"scr{col}")
                nc.vector.scalar_tensor_tensor(
                    out=s_scr[:],
                    in0=in0,
                    scalar=1.0,
                    in1=in1,
                    op0=Alu.mult,
                    op1=Alu.mult,
                    accum_out=s_pack[:, col : col + 1],
                )

            accum(s_wtn[:], s_pn[:], 4)                   # r1
            accum(s_wneg[:], s_rgb[:, 0:N], 0)            # -img_r
            accum(s_wneg[:], s_wtn[:], 5)                 # r3
            accum(s_wneg[:], s_invz[:], 3)                # -invdepth (invz last)

            # img g/b: product on Pool (gpsimd), sum on ACT (Copy + accum_out)
            for c in (1, 2):
                s_gp = scrp.tile([RS, N], F32, tag="gp", name=f"gp{c}")
                nc.gpsimd.tensor_mul(s_gp[:], s_wneg[:], s_rgb[:, c * N : (c + 1) * N])
                s_as = scrp.tile([RS, N], F32, tag="as", name=f"as{c}")
                nc.scalar.activation(
                    s_as[:],
                    s_gp[:],
                    AF.Copy,
                    bias=0.0,
                    scale=1.0,
                    accum_out=s_pack[:, c : c + 1],
                )

            nc.sync.dma_start(out=d_out[:], in_=s_pack[:])

    nc.compile()
    return nc


def _run(sigmas, rgbs, trace=False):
    """Shard, run on 8 cores, gather. Returns ((image, invdepth, l_dist), ns)."""
    global _BUILT
    if _BUILT is None:
        _BUILT = _build()
    nc = _BUILT
    from concourse.bass_utils import run_bass_kernel_spmd

    sig = np.ascontiguousarray(np.asarray(sigmas, dtype=np.float32))
    rgb = np.asarray(rgbs, dtype=np.float32)
    # channel-planar per ray: [r, c*N + k]
    rgbp = np.ascontiguousarray(rgb.transpose(0, 2, 1)).reshape(R, 3 * N)
    cst = _constants()

    in_maps = [
        {
            "sig": sig[i * RS : (i + 1) * RS],
            "rgb": rgbp[i * RS : (i + 1) * RS],
            "cst": cst,
        }
        for i in range(NCORES)
    ]
    # one retry: a crashed prior process can leave the NRT exec unit in a
    # transiently "unrecoverable" state that clears on the next attempt
    try:
        out = run_bass_kernel_spmd(nc, in_maps, list(range(NCORES)), trace=trace)
    except Exception:
        import time

        time.sleep(2.0)
        out = run_bass_kernel_spmd(nc, in_maps, list(range(NCORES)), trace=trace)
    res = np.concatenate(
        [out.results[i]["out"] for i in range(NCORES)], axis=0
    ).astype(np.float32)  # [1024, 9]

    image = (-res[:, 0:3]).astype(np.float32)      # [1024, 3]
    invdepth = (-res[:, 3]).astype(np.float32)     # [1024]
    r1, r3 = res[:, 4], res[:, 5]
    nW = res[:, 6] + res[:, 7]                     # -W split across two accums
    nT = res[:, 8]
    lray = 2.0 * (2.0 * r1 - nW * nT - r3)
    l_dist = np.float32(np.mean(lray, dtype=np.float64))
    return (image[None], invdepth[None], l_dist), out.exec_time_ns


def kernel(sigmas, rgbs):
    (image, invdepth, l_dist), _ = _run(sigmas, rgbs, trace=False)
    return image, invdepth, l_dist
